# revision 12
# baseline (speedup 1.0000x reference)
"""Trainium2 Bass kernel for NeuromodulatedAttention.

Key identity: the reference adds a per-query-row constant (dopamine-serotonin
modulation) to the scores and then standardizes each row ((m - mean)/(std+eps))
before the second softmax.  Adding a row constant shifts the row mean by the
same constant and leaves the row std unchanged, so the modulation cancels
exactly.  Row standardization is also invariant to the 1/sqrt(d) scale except
through the +1e-6 epsilon, which folds into eps' = sqrt(d)*1e-6 applied to the
raw-score std.  The kernel therefore computes:

    S   = Q @ K^T                       (raw scores)
    m   = (S - mean_k(S)) / (std_k(S, ddof=1) + sqrt(d)*1e-6)
    out = softmax_k(m) @ V

Sharding: 8 cores; core c handles batch b=c//2, query rows [h*1024,(h+1)*1024)
with h=c%2.  K/V of the batch are replicated per core; all ops batch-local.

Implementation notes:
 - f32r matmuls (full-rate fp32 path); Q^T/K^T via exact fp32 PE transposes.
 - scores per q-tile live briefly in 4 one-bank PSUM tiles and are evacuated
   to SBUF by ACT copies whose accumulators give the row-sums for free; the
   sumsq comes from one DVE scalar_tensor_tensor per bank.  Banks recycle
   immediately, so the next tiles' QK matmuls overlap the stats chain.
 - rsqrt for the row std: bit-trick seed + 2 Newton steps, fused into
   scalar_tensor_tensor ops; 1/rowsum via fast reciprocal + one Newton step.
 - P = exp(a*S+b) in one ACT pass (per-partition scale/bias, free row-sum);
   P^T via f32r PE transposes in groups of 4 sharing one PSUM bank.
 - Two q-tiles are processed per software-pipeline cycle so their serial
   stats chains interleave on DVE; emission order per cycle is
   [chain+exp(c); QK(c+1); P^T(c); PV(c)] which keeps the PE stream dense.
"""

import numpy as np

B, S, D = 4, 2048, 512
SQ = S // 2            # per-core query rows
P = 128                # partitions
QT_TILES = SQ // P     # 8 q-tiles per core
NPAIR = QT_TILES // 2  # tiles processed in pairs
KC = S // P            # 16 k-chunks
DC = D // P            # 4 d-chunks
NBANK = S // 512       # 4 psum bank-regions for scores

EPS = float(np.float32(np.sqrt(np.float32(512.0)))) * 1e-6  # sqrt(d)*1e-6
C1 = 1.0 / (S - 1)             # 1/2047
C2 = 1.0 / (S * (S - 1.0))    # 1/(2048*2047)
RSQRT_MAGIC = 0x5F3759DF

_CACHE = {}


def _build(use_f32r=True):
    import concourse.bacc as bacc
    import concourse.mybir as mybir
    import concourse.tile as tile
    from concourse.masks import make_identity

    f32 = mybir.dt.float32
    mmdt = mybir.dt.float32r if use_f32r else mybir.dt.float32
    u32 = mybir.dt.uint32
    AF = mybir.ActivationFunctionType
    OP = mybir.AluOpType

    nc = bacc.Bacc(None)
    Qd = nc.dram_tensor("Q", [SQ, D], f32, kind="ExternalInput")
    Kd = nc.dram_tensor("K", [S, D], f32, kind="ExternalInput")
    Vd = nc.dram_tensor("V", [S, D], f32, kind="ExternalInput")
    Od = nc.dram_tensor("out", [SQ, D], f32, kind="ExternalOutput")

    def copy_eng(i):
        return nc.vector if (i % 2 == 0) else nc.scalar

    def do_copy(eng, out_ap, in_ap):
        if eng is nc.vector:
            nc.vector.tensor_copy(out_ap, in_ap)
        else:
            nc.scalar.copy(out_ap, in_ap)

    with tile.TileContext(nc) as tc:
        with (
            tc.tile_pool(name="const", bufs=1) as constp,
            tc.tile_pool(name="ktc", bufs=1) as ktp,
            tc.tile_pool(name="vc", bufs=1) as vp,
            tc.tile_pool(name="stage", bufs=4) as stagep,
            tc.tile_pool(name="qt", bufs=4) as qtp,
            tc.tile_pool(name="pbuf", bufs=4) as pp,
            tc.tile_pool(name="sqp", bufs=2) as sqp,
            tc.tile_pool(name="ptb", bufs=3) as ptp,
            tc.tile_pool(name="osb", bufs=3) as op_,
            tc.tile_pool(name="vec", bufs=6) as vecp,
            tc.tile_pool(name="sps", bufs=1, space="PSUM") as sps,
            tc.tile_pool(name="tps", bufs=2, space="PSUM") as tps,
            tc.tile_pool(name="ops", bufs=2, space="PSUM") as ops,
        ):
            ident = constp.tile([P, P], f32)
            make_identity(nc, ident[:])
            ident_r = constp.tile([P, P], mmdt)
            nc.vector.tensor_copy(ident_r[:], ident[:])
            magic = constp.tile([P, 1], u32)
            nc.vector.memset(magic[:], RSQRT_MAGIC)

            # ---- K^T cache: KT[dlow, dc, k] = K[k, dc*128+dlow], f32r ----
            KT = ktp.tile([P, DC, S], mmdt)
            for kc in range(KC):
                kst = stagep.tile([P, D], f32, name="kst", tag="stage")
                nc.sync.dma_start(kst[:], Kd[kc * P:(kc + 1) * P, :])
                tg = tps.tile([P, DC, P], f32, name="ktg", tag="tp")
                for dc in range(DC):
                    nc.tensor.transpose(
                        tg[:, dc, :], kst[:, dc * P:(dc + 1) * P], ident[:]
                    )
                do_copy(copy_eng(kc), KT[:, :, kc * P:(kc + 1) * P], tg[:])

            # ---- V cache: one SWDGE casting DMA (fp32 -> f32r), no staging ----
            VC = vp.tile([P, KC, D], mmdt)

            def load_v():
                nc.gpsimd.dma_start(
                    VC[:], Vd.rearrange("(kc kl) d -> kl kc d", kl=P)
                )

            ev_t = [None] * QT_TILES     # evacuated scores (SBUF f32)
            s1_t = [None] * QT_TILES     # per-bank row-sum partials [P, NBANK]
            s2_t = [None] * QT_TILES     # per-bank row-sumsq partials
            pt_t = [None] * QT_TILES     # (Pt, rr)
            ptT_t = [None] * QT_TILES    # P^T tiles

            def phase_a1(c):
                """Q^T prep + QK + psum evacuation + sumsq for tiles 2c, 2c+1."""
                qts = []
                for t in (2 * c, 2 * c + 1):
                    qst = stagep.tile([P, D], f32, name="qst", tag="stage")
                    nc.sync.dma_start(qst[:], Qd[t * P:(t + 1) * P, :])
                    QTt = qtp.tile([P, DC, P], mmdt, name="QTt")
                    tg = tps.tile([P, DC, P], f32, name="qtg", tag="tp")
                    for dc in range(DC):
                        nc.tensor.transpose(
                            tg[:, dc, :], qst[:, dc * P:(dc + 1) * P], ident[:]
                        )
                    do_copy(copy_eng(t), QTt[:], tg[:])
                    qts.append(QTt)

                for ti, t in enumerate((2 * c, 2 * c + 1)):
                    QTt = qts[ti]
                    ev = pp.tile([P, S], f32, name="ev", tag="ev")
                    sqs = sqp.tile([P, S], f32, name="sqscratch", tag="sqscratch")
                    s1p = vecp.tile([P, NBANK], f32, name="s1p")
                    s2p = vecp.tile([P, NBANK], f32, name="s2p")
                    for bk in range(NBANK):
                        bsl = slice(bk * 512, (bk + 1) * 512)
                        sc = sps.tile([P, 512], f32, name=f"sc{bk}", tag=f"sc{bk}")
                        for dc in range(DC):
                            nc.tensor.matmul(
                                sc[:],
                                QTt[:, dc, :],
                                KT[:, dc, bsl],
                                start=(dc == 0),
                                stop=(dc == DC - 1),
                            )
                        nc.scalar.activation(
                            ev[:, bsl], sc[:], AF.Copy,
                            accum_out=s1p[:, bk:bk + 1],
                        )
                        nc.vector.scalar_tensor_tensor(
                            sqs[:, bsl], ev[:, bsl], 1.0, ev[:, bsl],
                            OP.mult, OP.mult, accum_out=s2p[:, bk:bk + 1],
                        )
                    ev_t[t], s1_t[t], s2_t[t] = ev, s1p, s2p

            def phase_a2(c):
                """Interleaved stats chains + exp for tiles 2c, 2c+1."""
                ts = (2 * c, 2 * c + 1)
                v = {}

                def vt(nm, t, dt=f32):
                    tl = vecp.tile([P, 1], dt, name=f"{nm}")
                    v[(nm, t)] = tl
                    return tl[:]

                for t in ts:
                    nc.vector.reduce_sum(
                        vt("s1", t), s1_t[t][:], axis=mybir.AxisListType.X
                    )
                for t in ts:
                    nc.vector.reduce_sum(
                        vt("s2", t), s2_t[t][:], axis=mybir.AxisListType.X
                    )
                for t in ts:  # t1s = (s1*C2)*s1
                    nc.vector.scalar_tensor_tensor(
                        vt("t1s", t), v[("s1", t)][:], C2, v[("s1", t)][:],
                        OP.mult, OP.mult,
                    )
                for t in ts:  # var = s2*C1 - t1s
                    nc.vector.scalar_tensor_tensor(
                        vt("var", t), v[("s2", t)][:], C1, v[("t1s", t)][:],
                        OP.mult, OP.subtract,
                    )
                for t in ts:  # bits = var >> 1
                    nc.vector.tensor_scalar(
                        vt("bits", t, u32), v[("var", t)][:].bitcast(u32), 1, None,
                        OP.logical_shift_right,
                    )
                for t in ts:  # y0 = magic - bits
                    nc.vector.tensor_tensor(
                        vt("y", t).bitcast(u32), magic[:], v[("bits", t)][:],
                        OP.subtract,
                    )
                for it in range(2):  # Newton: y = y*(1.5 - 0.5*var*y*y)
                    for t in ts:
                        if it == 0:
                            vt("t2", t)
                        nc.vector.scalar_tensor_tensor(
                            v[("t2", t)][:], v[("y", t)][:], 1.0, v[("y", t)][:],
                            OP.mult, OP.mult,
                        )
                    for t in ts:
                        nc.vector.scalar_tensor_tensor(
                            v[("t2", t)][:], v[("t2", t)][:], -0.5,
                            v[("var", t)][:], OP.mult, OP.mult,
                        )
                    for t in ts:
                        nc.vector.scalar_tensor_tensor(
                            v[("y", t)][:], v[("t2", t)][:], 1.5, v[("y", t)][:],
                            OP.add, OP.mult,
                        )
                for t in ts:  # t3 = -eps*y^2 ; a = y + t3
                    nc.vector.scalar_tensor_tensor(
                        vt("t3", t), v[("y", t)][:], -EPS, v[("y", t)][:],
                        OP.mult, OP.mult,
                    )
                for t in ts:
                    nc.vector.tensor_tensor(
                        vt("a", t), v[("y", t)][:], v[("t3", t)][:], OP.add
                    )
                for t in ts:  # bb = (s1 * -1/S) * a
                    nc.vector.scalar_tensor_tensor(
                        vt("bb", t), v[("s1", t)][:], -1.0 / S, v[("a", t)][:],
                        OP.mult, OP.mult,
                    )

                for t in ts:
                    Pt = pp.tile([P, S], mmdt, name="Pt", tag="Pt")
                    rs = vt("rs", t)
                    nc.scalar.activation(
                        Pt[:], ev_t[t][:], AF.Exp,
                        bias=v[("bb", t)][:], scale=v[("a", t)][:],
                        accum_out=rs,
                    )
                    rr = vt("rr", t)
                    scr = vt("scr", t)
                    nc.vector.reciprocal_approx_fast(out=scr, in_=rs)
                    # one Newton step: rr = (2 - rs*scr) * scr
                    rn = vt("rn", t)
                    nc.vector.scalar_tensor_tensor(
                        rn, rs, 1.0, scr, OP.mult, OP.mult
                    )
                    nc.vector.tensor_scalar(rn, rn, -1.0, 2.0, OP.mult, OP.add)
                    nc.vector.tensor_tensor(rr, rn, scr, OP.mult)
                    pt_t[t] = (Pt, rr)

            def phase_b1(c):
                for t in (2 * c, 2 * c + 1):
                    Pt, _ = pt_t[t]
                    PT = ptp.tile([P, KC, P], mmdt, name="PT")
                    for g in range(KC // DC):
                        tg2 = tps.tile([P, DC, P], mmdt, name="ptg", tag="tp")
                        for j in range(DC):
                            kc = g * DC + j
                            nc.tensor.transpose(
                                tg2[:, j, :], Pt[:, kc * P:(kc + 1) * P],
                                ident_r[:],
                            )
                        do_copy(
                            copy_eng(g + t), PT[:, g * DC:(g + 1) * DC, :], tg2[:]
                        )
                    ptT_t[t] = PT

            def phase_b2(c):
                for t in (2 * c, 2 * c + 1):
                    _, rr = pt_t[t]
                    PT = ptT_t[t]
                    ot = ops.tile([P, D], f32, name="ot")
                    for kc in range(KC):
                        nc.tensor.matmul(
                            ot[:],
                            PT[:, kc, :],
                            VC[:, kc, :],
                            start=(kc == 0),
                            stop=(kc == KC - 1),
                        )
                    osb = op_.tile([P, D], f32, name="osb")
                    nc.scalar.mul(osb[:], ot[:], rr[:])
                    nc.sync.dma_start(Od[t * P:(t + 1) * P, :], osb[:])

            phase_a1(0)
            load_v()
            for c in range(NPAIR):
                phase_a2(c)
                if c + 1 < NPAIR:
                    phase_a1(c + 1)
                phase_b1(c)
                phase_b2(c)

    nc.compile()
    return nc


def _get_nc():
    if "nc" not in _CACHE:
        _CACHE["nc"] = _build(use_f32r=True)
    return _CACHE["nc"]


def _shard(Q, K, V):
    Q = np.ascontiguousarray(np.asarray(Q, dtype=np.float32))
    K = np.ascontiguousarray(np.asarray(K, dtype=np.float32))
    V = np.ascontiguousarray(np.asarray(V, dtype=np.float32))
    in_maps = []
    for c in range(8):
        b, h = divmod(c, 2)
        in_maps.append(
            {
                "Q": np.ascontiguousarray(Q[b, h * SQ:(h + 1) * SQ]),
                "K": np.ascontiguousarray(K[b]),
                "V": np.ascontiguousarray(V[b]),
            }
        )
    return in_maps


def _gather(results):
    out = np.empty((B, S, D), np.float32)
    for c in range(8):
        b, h = divmod(c, 2)
        out[b, h * SQ:(h + 1) * SQ] = results[c]["out"]
    return out


def kernel(Q, K, V, **_unused):
    from concourse.bass_utils import run_bass_kernel_spmd

    nc = _get_nc()
    res = run_bass_kernel_spmd(nc, _shard(Q, K, V), core_ids=list(range(8)))
    return _gather(res.results)


def kernel_traced(Q, K, V, tmpdir=None, **_unused):
    """Like kernel() but with NTFF profiling; returns (out, exec_time_ns)."""
    from concourse.bass_utils import run_bass_kernel_spmd

    nc = _get_nc()
    try:
        res = run_bass_kernel_spmd(
            nc, _shard(Q, K, V), core_ids=list(range(8)), trace=True, tmpdir=tmpdir
        )
    except ModuleNotFoundError:
        res = run_bass_kernel_spmd(nc, _shard(Q, K, V), core_ids=list(range(8)))
    return _gather(res.results), res.exec_time_ns


# revision 13
# speedup vs baseline: 1.0044x; 1.0044x over previous
"""Trainium2 Bass kernel for NeuromodulatedAttention.

Key identity: the reference adds a per-query-row constant (dopamine-serotonin
modulation) to the scores and then standardizes each row ((m - mean)/(std+eps))
before the second softmax.  Adding a row constant shifts the row mean by the
same constant and leaves the row std unchanged, so the modulation cancels
exactly.  Row standardization is also invariant to the 1/sqrt(d) scale except
through the +1e-6 epsilon, which folds into eps' = sqrt(d)*1e-6 applied to the
raw-score std.  The kernel therefore computes:

    S   = Q @ K^T                       (raw scores)
    m   = (S - mean_k(S)) / (std_k(S, ddof=1) + sqrt(d)*1e-6)
    out = softmax_k(m) @ V

Sharding: 8 cores; core c handles batch b=c//2, query rows [h*1024,(h+1)*1024)
with h=c%2.  K/V of the batch are replicated per core; all ops batch-local.

Implementation notes:
 - f32r matmuls (full-rate fp32 path); Q^T/K^T via exact fp32 PE transposes.
 - scores per q-tile live briefly in 4 one-bank PSUM tiles and are evacuated
   to SBUF by ACT copies whose accumulators give the row-sums for free; the
   sumsq comes from one DVE scalar_tensor_tensor per bank.  Banks recycle
   immediately, so the next tiles' QK matmuls overlap the stats chain.
 - rsqrt for the row std: bit-trick seed + 2 Newton steps, fused into
   scalar_tensor_tensor ops; 1/rowsum via fast reciprocal + one Newton step.
 - P = exp(a*S+b) in one ACT pass (per-partition scale/bias, free row-sum);
   P^T via f32r PE transposes in groups of 4 sharing one PSUM bank.
 - Two q-tiles are processed per software-pipeline cycle so their serial
   stats chains interleave on DVE; emission order per cycle is
   [chain+exp(c); QK(c+1); P^T(c); PV(c)] which keeps the PE stream dense.
"""

import numpy as np

B, S, D = 4, 2048, 512
SQ = S // 2            # per-core query rows
P = 128                # partitions
QT_TILES = SQ // P     # 8 q-tiles per core
NPAIR = QT_TILES // 2  # tiles processed in pairs
KC = S // P            # 16 k-chunks
DC = D // P            # 4 d-chunks
NBANK = S // 512       # 4 psum bank-regions for scores

EPS = float(np.float32(np.sqrt(np.float32(512.0)))) * 1e-6  # sqrt(d)*1e-6
C1 = 1.0 / (S - 1)             # 1/2047
C2 = 1.0 / (S * (S - 1.0))    # 1/(2048*2047)
RSQRT_MAGIC = 0x5F3759DF

_CACHE = {}


def _build(use_f32r=True):
    import concourse.bacc as bacc
    import concourse.mybir as mybir
    import concourse.tile as tile
    from concourse.masks import make_identity

    f32 = mybir.dt.float32
    mmdt = mybir.dt.float32r if use_f32r else mybir.dt.float32
    u32 = mybir.dt.uint32
    AF = mybir.ActivationFunctionType
    OP = mybir.AluOpType

    nc = bacc.Bacc(None)
    Qd = nc.dram_tensor("Q", [SQ, D], f32, kind="ExternalInput")
    Kd = nc.dram_tensor("K", [S, D], f32, kind="ExternalInput")
    Vd = nc.dram_tensor("V", [S, D], f32, kind="ExternalInput")
    Od = nc.dram_tensor("out", [SQ, D], f32, kind="ExternalOutput")

    def copy_eng(i):
        return nc.vector if (i % 2 == 0) else nc.scalar

    def do_copy(eng, out_ap, in_ap):
        if eng is nc.vector:
            nc.vector.tensor_copy(out_ap, in_ap)
        else:
            nc.scalar.copy(out_ap, in_ap)

    with tile.TileContext(nc) as tc:
        with (
            tc.tile_pool(name="const", bufs=1) as constp,
            tc.tile_pool(name="ktc", bufs=1) as ktp,
            tc.tile_pool(name="vc", bufs=1) as vp,
            tc.tile_pool(name="stage", bufs=6) as stagep,
            tc.tile_pool(name="qt", bufs=4) as qtp,
            tc.tile_pool(name="pbuf", bufs=4) as pp,
            tc.tile_pool(name="sqp", bufs=2) as sqp,
            tc.tile_pool(name="ptb", bufs=3) as ptp,
            tc.tile_pool(name="osb", bufs=3) as op_,
            tc.tile_pool(name="vec", bufs=6) as vecp,
            tc.tile_pool(name="sps", bufs=1, space="PSUM") as sps,
            tc.tile_pool(name="tps", bufs=2, space="PSUM") as tps,
            tc.tile_pool(name="ops", bufs=2, space="PSUM") as ops,
        ):
            ident = constp.tile([P, P], f32)
            make_identity(nc, ident[:])
            ident_r = constp.tile([P, P], mmdt)
            nc.vector.tensor_copy(ident_r[:], ident[:])
            magic = constp.tile([P, 1], u32)
            nc.vector.memset(magic[:], RSQRT_MAGIC)

            # ---- K^T cache, split per 512-wide bank so the first QK matmuls
            # only wait for the first 4 K row-chunks: KTb[b][dlow, dc, k'] ----
            KTb = [
                ktp.tile([P, DC, 512], mmdt, name=f"KT{b}", tag=f"KT{b}")
                for b in range(NBANK)
            ]
            for kc in range(KC):
                kst = stagep.tile([P, D], f32, name="kst", tag="stage")
                nc.sync.dma_start(kst[:], Kd[kc * P:(kc + 1) * P, :])
                tg = tps.tile([P, DC, P], f32, name="ktg", tag="tp")
                for dc in range(DC):
                    nc.tensor.transpose(
                        tg[:, dc, :], kst[:, dc * P:(dc + 1) * P], ident[:]
                    )
                ksub = kc % 4
                do_copy(
                    copy_eng(kc),
                    KTb[kc // 4][:, :, ksub * P:(ksub + 1) * P],
                    tg[:],
                )

            # ---- V cache: one SWDGE casting DMA (fp32 -> f32r), no staging ----
            VC = vp.tile([P, KC, D], mmdt)

            def load_v():
                nc.gpsimd.dma_start(
                    VC[:], Vd.rearrange("(kc kl) d -> kl kc d", kl=P)
                )

            ev_t = [None] * QT_TILES     # evacuated scores (SBUF f32)
            s1_t = [None] * QT_TILES     # per-bank row-sum partials [P, NBANK]
            s2_t = [None] * QT_TILES     # per-bank row-sumsq partials
            pt_t = [None] * QT_TILES     # (Pt, rr)
            ptT_t = [None] * QT_TILES    # P^T tiles

            def phase_a1(c):
                """Q^T prep + QK + psum evacuation + sumsq for tiles 2c, 2c+1."""
                qts = []
                for t in (2 * c, 2 * c + 1):
                    qst = stagep.tile([P, D], f32, name="qst", tag="stage")
                    nc.sync.dma_start(qst[:], Qd[t * P:(t + 1) * P, :])
                    QTt = qtp.tile([P, DC, P], mmdt, name="QTt")
                    tg = tps.tile([P, DC, P], f32, name="qtg", tag="tp")
                    for dc in range(DC):
                        nc.tensor.transpose(
                            tg[:, dc, :], qst[:, dc * P:(dc + 1) * P], ident[:]
                        )
                    do_copy(copy_eng(t), QTt[:], tg[:])
                    qts.append(QTt)

                for ti, t in enumerate((2 * c, 2 * c + 1)):
                    QTt = qts[ti]
                    ev = pp.tile([P, S], f32, name="ev", tag="ev")
                    sqs = sqp.tile([P, S], f32, name="sqscratch", tag="sqscratch")
                    s1p = vecp.tile([P, NBANK], f32, name="s1p")
                    s2p = vecp.tile([P, NBANK], f32, name="s2p")
                    for bk in range(NBANK):
                        bsl = slice(bk * 512, (bk + 1) * 512)
                        sc = sps.tile([P, 512], f32, name=f"sc{bk}", tag=f"sc{bk}")
                        for dc in range(DC):
                            nc.tensor.matmul(
                                sc[:],
                                QTt[:, dc, :],
                                KTb[bk][:, dc, :],
                                start=(dc == 0),
                                stop=(dc == DC - 1),
                            )
                        nc.scalar.activation(
                            ev[:, bsl], sc[:], AF.Copy,
                            accum_out=s1p[:, bk:bk + 1],
                        )
                        nc.vector.scalar_tensor_tensor(
                            sqs[:, bsl], ev[:, bsl], 1.0, ev[:, bsl],
                            OP.mult, OP.mult, accum_out=s2p[:, bk:bk + 1],
                        )
                    ev_t[t], s1_t[t], s2_t[t] = ev, s1p, s2p

            def phase_a2(c):
                """Interleaved stats chains + exp for tiles 2c, 2c+1."""
                ts = (2 * c, 2 * c + 1)
                v = {}

                def vt(nm, t, dt=f32):
                    tl = vecp.tile([P, 1], dt, name=f"{nm}")
                    v[(nm, t)] = tl
                    return tl[:]

                for t in ts:
                    nc.vector.reduce_sum(
                        vt("s1", t), s1_t[t][:], axis=mybir.AxisListType.X
                    )
                for t in ts:
                    nc.vector.reduce_sum(
                        vt("s2", t), s2_t[t][:], axis=mybir.AxisListType.X
                    )
                for t in ts:  # t1s = (s1*C2)*s1
                    nc.vector.scalar_tensor_tensor(
                        vt("t1s", t), v[("s1", t)][:], C2, v[("s1", t)][:],
                        OP.mult, OP.mult,
                    )
                for t in ts:  # var = s2*C1 - t1s
                    nc.vector.scalar_tensor_tensor(
                        vt("var", t), v[("s2", t)][:], C1, v[("t1s", t)][:],
                        OP.mult, OP.subtract,
                    )
                for t in ts:  # bits = var >> 1
                    nc.vector.tensor_scalar(
                        vt("bits", t, u32), v[("var", t)][:].bitcast(u32), 1, None,
                        OP.logical_shift_right,
                    )
                for t in ts:  # y0 = magic - bits
                    nc.vector.tensor_tensor(
                        vt("y", t).bitcast(u32), magic[:], v[("bits", t)][:],
                        OP.subtract,
                    )
                for it in range(2):  # Newton: y = y*(1.5 - 0.5*var*y*y)
                    for t in ts:
                        if it == 0:
                            vt("t2", t)
                        nc.vector.scalar_tensor_tensor(
                            v[("t2", t)][:], v[("y", t)][:], 1.0, v[("y", t)][:],
                            OP.mult, OP.mult,
                        )
                    for t in ts:
                        nc.vector.scalar_tensor_tensor(
                            v[("t2", t)][:], v[("t2", t)][:], -0.5,
                            v[("var", t)][:], OP.mult, OP.mult,
                        )
                    for t in ts:
                        nc.vector.scalar_tensor_tensor(
                            v[("y", t)][:], v[("t2", t)][:], 1.5, v[("y", t)][:],
                            OP.add, OP.mult,
                        )
                for t in ts:  # t3 = -eps*y^2 ; a = y + t3
                    nc.vector.scalar_tensor_tensor(
                        vt("t3", t), v[("y", t)][:], -EPS, v[("y", t)][:],
                        OP.mult, OP.mult,
                    )
                for t in ts:
                    nc.vector.tensor_tensor(
                        vt("a", t), v[("y", t)][:], v[("t3", t)][:], OP.add
                    )
                for t in ts:  # bb = (s1 * -1/S) * a
                    nc.vector.scalar_tensor_tensor(
                        vt("bb", t), v[("s1", t)][:], -1.0 / S, v[("a", t)][:],
                        OP.mult, OP.mult,
                    )

                for t in ts:
                    Pt = pp.tile([P, S], mmdt, name="Pt", tag="Pt")
                    rs = vt("rs", t)
                    nc.scalar.activation(
                        Pt[:], ev_t[t][:], AF.Exp,
                        bias=v[("bb", t)][:], scale=v[("a", t)][:],
                        accum_out=rs,
                    )
                    rr = vt("rr", t)
                    scr = vt("scr", t)
                    nc.vector.reciprocal_approx_fast(out=scr, in_=rs)
                    # one Newton step: rr = (2 - rs*scr) * scr
                    rn = vt("rn", t)
                    nc.vector.scalar_tensor_tensor(
                        rn, rs, 1.0, scr, OP.mult, OP.mult
                    )
                    nc.vector.tensor_scalar(rn, rn, -1.0, 2.0, OP.mult, OP.add)
                    nc.vector.tensor_tensor(rr, rn, scr, OP.mult)
                    pt_t[t] = (Pt, rr)

            def phase_b1(c):
                for t in (2 * c, 2 * c + 1):
                    Pt, _ = pt_t[t]
                    PT = ptp.tile([P, KC, P], mmdt, name="PT")
                    for g in range(KC // DC):
                        tg2 = tps.tile([P, DC, P], mmdt, name="ptg", tag="tp")
                        for j in range(DC):
                            kc = g * DC + j
                            nc.tensor.transpose(
                                tg2[:, j, :], Pt[:, kc * P:(kc + 1) * P],
                                ident_r[:],
                            )
                        do_copy(
                            copy_eng(g + t), PT[:, g * DC:(g + 1) * DC, :], tg2[:]
                        )
                    ptT_t[t] = PT

            def phase_b2(c):
                for t in (2 * c, 2 * c + 1):
                    _, rr = pt_t[t]
                    PT = ptT_t[t]
                    ot = ops.tile([P, D], f32, name="ot")
                    for kc in range(KC):
                        nc.tensor.matmul(
                            ot[:],
                            PT[:, kc, :],
                            VC[:, kc, :],
                            start=(kc == 0),
                            stop=(kc == KC - 1),
                        )
                    osb = op_.tile([P, D], f32, name="osb")
                    nc.scalar.mul(osb[:], ot[:], rr[:])
                    nc.sync.dma_start(Od[t * P:(t + 1) * P, :], osb[:])

            phase_a1(0)
            load_v()
            for c in range(NPAIR):
                phase_a2(c)
                if c + 1 < NPAIR:
                    phase_a1(c + 1)
                phase_b1(c)
                phase_b2(c)

    nc.compile()
    return nc


def _get_nc():
    if "nc" not in _CACHE:
        _CACHE["nc"] = _build(use_f32r=True)
    return _CACHE["nc"]


def _shard(Q, K, V):
    Q = np.ascontiguousarray(np.asarray(Q, dtype=np.float32))
    K = np.ascontiguousarray(np.asarray(K, dtype=np.float32))
    V = np.ascontiguousarray(np.asarray(V, dtype=np.float32))
    in_maps = []
    for c in range(8):
        b, h = divmod(c, 2)
        in_maps.append(
            {
                "Q": np.ascontiguousarray(Q[b, h * SQ:(h + 1) * SQ]),
                "K": np.ascontiguousarray(K[b]),
                "V": np.ascontiguousarray(V[b]),
            }
        )
    return in_maps


def _gather(results):
    out = np.empty((B, S, D), np.float32)
    for c in range(8):
        b, h = divmod(c, 2)
        out[b, h * SQ:(h + 1) * SQ] = results[c]["out"]
    return out


def kernel(Q, K, V, **_unused):
    from concourse.bass_utils import run_bass_kernel_spmd

    nc = _get_nc()
    res = run_bass_kernel_spmd(nc, _shard(Q, K, V), core_ids=list(range(8)))
    return _gather(res.results)


def kernel_traced(Q, K, V, tmpdir=None, **_unused):
    """Like kernel() but with NTFF profiling; returns (out, exec_time_ns)."""
    from concourse.bass_utils import run_bass_kernel_spmd

    nc = _get_nc()
    try:
        res = run_bass_kernel_spmd(
            nc, _shard(Q, K, V), core_ids=list(range(8)), trace=True, tmpdir=tmpdir
        )
    except ModuleNotFoundError:
        res = run_bass_kernel_spmd(nc, _shard(Q, K, V), core_ids=list(range(8)))
    return _gather(res.results), res.exec_time_ns


# revision 14
# speedup vs baseline: 1.0683x; 1.0637x over previous
"""Trainium2 Bass kernel for NeuromodulatedAttention.

Key identity: the reference adds a per-query-row constant (dopamine-serotonin
modulation) to the scores and then standardizes each row ((m - mean)/(std+eps))
before the second softmax.  Adding a row constant shifts the row mean by the
same constant and leaves the row std unchanged, so the modulation cancels
exactly.  Row standardization is also invariant to the 1/sqrt(d) scale except
through the +1e-6 epsilon, which folds into eps' = sqrt(d)*1e-6 applied to the
raw-score std.  The kernel therefore computes:

    S   = Q @ K^T                       (raw scores)
    m   = (S - mean_k(S)) / (std_k(S, ddof=1) + sqrt(d)*1e-6)
    out = softmax_k(m) @ V

Sharding: 8 cores; core c handles batch b=c//2, query rows [h*1024,(h+1)*1024)
with h=c%2.  K/V of the batch are replicated per core; all ops batch-local.

Implementation notes:
 - f32r matmuls (full-rate fp32 path); Q^T/K^T via exact fp32 PE transposes.
 - scores per q-tile live briefly in 4 one-bank PSUM tiles and are evacuated
   to SBUF by ACT copies whose accumulators give the row-sums for free; the
   sumsq comes from one DVE scalar_tensor_tensor per bank.  Banks recycle
   immediately, so the next tiles' QK matmuls overlap the stats chain.
 - rsqrt for the row std: bit-trick seed + 2 Newton steps, fused into
   scalar_tensor_tensor ops; 1/rowsum via fast reciprocal + one Newton step.
 - P = exp(a*S+b) in one ACT pass (per-partition scale/bias, free row-sum);
   P^T via f32r PE transposes in groups of 4 sharing one PSUM bank.
 - Two q-tiles are processed per software-pipeline cycle so their serial
   stats chains interleave on DVE; emission order per cycle is
   [chain+exp(c); QK(c+1); P^T(c); PV(c)] which keeps the PE stream dense.
"""

import numpy as np

B, S, D = 4, 2048, 512
SQ = S // 2            # per-core query rows
P = 128                # partitions
QT_TILES = SQ // P     # 8 q-tiles per core
NPAIR = QT_TILES // 2  # tiles processed in pairs
KC = S // P            # 16 k-chunks
DC = D // P            # 4 d-chunks
NBANK = S // 512       # 4 psum bank-regions for scores

EPS = float(np.float32(np.sqrt(np.float32(512.0)))) * 1e-6  # sqrt(d)*1e-6
C1 = 1.0 / (S - 1)             # 1/2047
C2 = 1.0 / (S * (S - 1.0))    # 1/(2048*2047)
RSQRT_MAGIC = 0x5F3759DF

_CACHE = {}


def _build(use_f32r=True):
    import concourse.bacc as bacc
    import concourse.mybir as mybir
    import concourse.tile as tile
    from concourse.masks import make_identity

    f32 = mybir.dt.float32
    mmdt = mybir.dt.float32r if use_f32r else mybir.dt.float32
    u32 = mybir.dt.uint32
    AF = mybir.ActivationFunctionType
    OP = mybir.AluOpType

    nc = bacc.Bacc(None)
    Qd = nc.dram_tensor("Q", [SQ, D], f32, kind="ExternalInput")
    Kd = nc.dram_tensor("K", [S, D], f32, kind="ExternalInput")
    Vd = nc.dram_tensor("V", [S, D], f32, kind="ExternalInput")
    Od = nc.dram_tensor("out", [SQ, D], f32, kind="ExternalOutput")

    def copy_eng(i):
        return nc.vector if (i % 2 == 0) else nc.scalar

    def do_copy(eng, out_ap, in_ap):
        if eng is nc.vector:
            nc.vector.tensor_copy(out_ap, in_ap)
        else:
            nc.scalar.copy(out_ap, in_ap)

    with tile.TileContext(nc) as tc:
        with (
            tc.tile_pool(name="const", bufs=1) as constp,
            tc.tile_pool(name="ktc", bufs=1) as ktp,
            tc.tile_pool(name="vc", bufs=1) as vp,
            tc.tile_pool(name="stage", bufs=6) as stagep,
            tc.tile_pool(name="qt", bufs=4) as qtp,
            tc.tile_pool(name="pbuf", bufs=4) as pp,
            tc.tile_pool(name="sqp", bufs=2) as sqp,
            tc.tile_pool(name="ptb", bufs=3) as ptp,
            tc.tile_pool(name="osb", bufs=3) as op_,
            tc.tile_pool(name="vec", bufs=6) as vecp,
            tc.tile_pool(name="bank6", bufs=6, space="PSUM") as bk6,
            tc.tile_pool(name="ops", bufs=2, space="PSUM") as ops,
        ):
            ident = constp.tile([P, P], f32)
            make_identity(nc, ident[:])
            ident_r = constp.tile([P, P], mmdt)
            nc.vector.tensor_copy(ident_r[:], ident[:])
            magic = constp.tile([P, 1], u32)
            nc.vector.memset(magic[:], RSQRT_MAGIC)

            # ---- K^T cache, split per 512-wide bank so the first QK matmuls
            # only wait for the first 4 K row-chunks: KTb[b][dlow, dc, k'] ----
            KTb = [
                ktp.tile([P, DC, 512], mmdt, name=f"KT{b}", tag=f"KT{b}")
                for b in range(NBANK)
            ]
            for kc in range(KC):
                kst = stagep.tile([P, D], f32, name="kst", tag="stage")
                nc.sync.dma_start(kst[:], Kd[kc * P:(kc + 1) * P, :])
                tg = bk6.tile([P, DC, P], f32, name="ktg", tag="bk")
                for dc in range(DC):
                    nc.tensor.transpose(
                        tg[:, dc, :], kst[:, dc * P:(dc + 1) * P], ident[:]
                    )
                ksub = kc % 4
                do_copy(
                    copy_eng(kc),
                    KTb[kc // 4][:, :, ksub * P:(ksub + 1) * P],
                    tg[:],
                )

            # ---- V cache: one SWDGE casting DMA (fp32 -> f32r), no staging ----
            VC = vp.tile([P, KC, D], mmdt)

            def load_v():
                nc.gpsimd.dma_start(
                    VC[:], Vd.rearrange("(kc kl) d -> kl kc d", kl=P)
                )

            ev_t = [None] * QT_TILES     # evacuated scores (SBUF f32)
            s1_t = [None] * QT_TILES     # per-bank row-sum partials [P, NBANK]
            s2_t = [None] * QT_TILES     # per-bank row-sumsq partials
            pt_t = [None] * QT_TILES     # (Pt, rr)
            ptT_t = [None] * QT_TILES    # P^T tiles

            def phase_a1(c):
                """Q^T prep + QK + psum evacuation + sumsq for tiles 2c, 2c+1."""
                qts = []
                for t in (2 * c, 2 * c + 1):
                    qst = stagep.tile([P, D], f32, name="qst", tag="stage")
                    nc.sync.dma_start(qst[:], Qd[t * P:(t + 1) * P, :])
                    QTt = qtp.tile([P, DC, P], mmdt, name="QTt")
                    tg = bk6.tile([P, DC, P], f32, name="qtg", tag="bk")
                    for dc in range(DC):
                        nc.tensor.transpose(
                            tg[:, dc, :], qst[:, dc * P:(dc + 1) * P], ident[:]
                        )
                    do_copy(copy_eng(t), QTt[:], tg[:])
                    qts.append(QTt)

                for ti, t in enumerate((2 * c, 2 * c + 1)):
                    QTt = qts[ti]
                    ev = pp.tile([P, S], f32, name="ev", tag="ev")
                    sqs = sqp.tile([P, S], f32, name="sqscratch", tag="sqscratch")
                    s1p = vecp.tile([P, NBANK], f32, name="s1p")
                    s2p = vecp.tile([P, NBANK], f32, name="s2p")
                    for bk in range(NBANK):
                        bsl = slice(bk * 512, (bk + 1) * 512)
                        sc = bk6.tile([P, 512], f32, name=f"sc{bk}", tag="bk")
                        for dc in range(DC):
                            nc.tensor.matmul(
                                sc[:],
                                QTt[:, dc, :],
                                KTb[bk][:, dc, :],
                                start=(dc == 0),
                                stop=(dc == DC - 1),
                            )
                        nc.scalar.activation(
                            ev[:, bsl], sc[:], AF.Copy,
                            accum_out=s1p[:, bk:bk + 1],
                        )
                        nc.vector.scalar_tensor_tensor(
                            sqs[:, bsl], ev[:, bsl], 1.0, ev[:, bsl],
                            OP.mult, OP.mult, accum_out=s2p[:, bk:bk + 1],
                        )
                    ev_t[t], s1_t[t], s2_t[t] = ev, s1p, s2p

            def phase_a2(c):
                """Interleaved stats chains + exp for tiles 2c, 2c+1."""
                ts = (2 * c, 2 * c + 1)
                v = {}

                def vt(nm, t, dt=f32):
                    tl = vecp.tile([P, 1], dt, name=f"{nm}")
                    v[(nm, t)] = tl
                    return tl[:]

                for t in ts:
                    nc.vector.reduce_sum(
                        vt("s1", t), s1_t[t][:], axis=mybir.AxisListType.X
                    )
                for t in ts:
                    nc.vector.reduce_sum(
                        vt("s2", t), s2_t[t][:], axis=mybir.AxisListType.X
                    )
                for t in ts:  # t1s = (s1*C2)*s1
                    nc.vector.scalar_tensor_tensor(
                        vt("t1s", t), v[("s1", t)][:], C2, v[("s1", t)][:],
                        OP.mult, OP.mult,
                    )
                for t in ts:  # var = s2*C1 - t1s
                    nc.vector.scalar_tensor_tensor(
                        vt("var", t), v[("s2", t)][:], C1, v[("t1s", t)][:],
                        OP.mult, OP.subtract,
                    )
                for t in ts:  # bits = var >> 1
                    nc.vector.tensor_scalar(
                        vt("bits", t, u32), v[("var", t)][:].bitcast(u32), 1, None,
                        OP.logical_shift_right,
                    )
                for t in ts:  # y0 = magic - bits
                    nc.vector.tensor_tensor(
                        vt("y", t).bitcast(u32), magic[:], v[("bits", t)][:],
                        OP.subtract,
                    )
                for it in range(2):  # Newton: y = y*(1.5 - 0.5*var*y*y)
                    for t in ts:
                        if it == 0:
                            vt("t2", t)
                        nc.vector.scalar_tensor_tensor(
                            v[("t2", t)][:], v[("y", t)][:], 1.0, v[("y", t)][:],
                            OP.mult, OP.mult,
                        )
                    for t in ts:
                        nc.vector.scalar_tensor_tensor(
                            v[("t2", t)][:], v[("t2", t)][:], -0.5,
                            v[("var", t)][:], OP.mult, OP.mult,
                        )
                    for t in ts:
                        nc.vector.scalar_tensor_tensor(
                            v[("y", t)][:], v[("t2", t)][:], 1.5, v[("y", t)][:],
                            OP.add, OP.mult,
                        )
                for t in ts:  # t3 = -eps*y^2 ; a = y + t3
                    nc.vector.scalar_tensor_tensor(
                        vt("t3", t), v[("y", t)][:], -EPS, v[("y", t)][:],
                        OP.mult, OP.mult,
                    )
                for t in ts:
                    nc.vector.tensor_tensor(
                        vt("a", t), v[("y", t)][:], v[("t3", t)][:], OP.add
                    )
                for t in ts:  # bb = (s1 * -1/S) * a
                    nc.vector.scalar_tensor_tensor(
                        vt("bb", t), v[("s1", t)][:], -1.0 / S, v[("a", t)][:],
                        OP.mult, OP.mult,
                    )

                for t in ts:
                    Pt = pp.tile([P, S], mmdt, name="Pt", tag="Pt")
                    rs = vt("rs", t)
                    nc.scalar.activation(
                        Pt[:], ev_t[t][:], AF.Exp,
                        bias=v[("bb", t)][:], scale=v[("a", t)][:],
                        accum_out=rs,
                    )
                    rr = vt("rr", t)
                    scr = vt("scr", t)
                    nc.vector.reciprocal_approx_fast(out=scr, in_=rs)
                    # one Newton step: rr = (2 - rs*scr) * scr
                    rn = vt("rn", t)
                    nc.vector.scalar_tensor_tensor(
                        rn, rs, 1.0, scr, OP.mult, OP.mult
                    )
                    nc.vector.tensor_scalar(rn, rn, -1.0, 2.0, OP.mult, OP.add)
                    nc.vector.tensor_tensor(rr, rn, scr, OP.mult)
                    pt_t[t] = (Pt, rr)

            def phase_b1(c):
                for t in (2 * c, 2 * c + 1):
                    Pt, _ = pt_t[t]
                    PT = ptp.tile([P, KC, P], mmdt, name="PT")
                    for g in range(KC // DC):
                        tg2 = bk6.tile([P, DC, P], mmdt, name="ptg", tag="bk")
                        for j in range(DC):
                            kc = g * DC + j
                            nc.tensor.transpose(
                                tg2[:, j, :], Pt[:, kc * P:(kc + 1) * P],
                                ident_r[:],
                            )
                        do_copy(
                            copy_eng(g + t), PT[:, g * DC:(g + 1) * DC, :], tg2[:]
                        )
                    ptT_t[t] = PT

            def phase_b2(c):
                for t in (2 * c, 2 * c + 1):
                    _, rr = pt_t[t]
                    PT = ptT_t[t]
                    ot = ops.tile([P, D], f32, name="ot")
                    for kc in range(KC):
                        nc.tensor.matmul(
                            ot[:],
                            PT[:, kc, :],
                            VC[:, kc, :],
                            start=(kc == 0),
                            stop=(kc == KC - 1),
                        )
                    osb = op_.tile([P, D], f32, name="osb")
                    nc.scalar.mul(osb[:], ot[:], rr[:])
                    nc.sync.dma_start(Od[t * P:(t + 1) * P, :], osb[:])

            phase_a1(0)
            load_v()
            for c in range(NPAIR):
                phase_a2(c)
                if c + 1 < NPAIR:
                    phase_a1(c + 1)
                phase_b1(c)
                phase_b2(c)

    nc.compile()
    return nc


def _get_nc():
    if "nc" not in _CACHE:
        _CACHE["nc"] = _build(use_f32r=True)
    return _CACHE["nc"]


def _shard(Q, K, V):
    Q = np.ascontiguousarray(np.asarray(Q, dtype=np.float32))
    K = np.ascontiguousarray(np.asarray(K, dtype=np.float32))
    V = np.ascontiguousarray(np.asarray(V, dtype=np.float32))
    in_maps = []
    for c in range(8):
        b, h = divmod(c, 2)
        in_maps.append(
            {
                "Q": np.ascontiguousarray(Q[b, h * SQ:(h + 1) * SQ]),
                "K": np.ascontiguousarray(K[b]),
                "V": np.ascontiguousarray(V[b]),
            }
        )
    return in_maps


def _gather(results):
    out = np.empty((B, S, D), np.float32)
    for c in range(8):
        b, h = divmod(c, 2)
        out[b, h * SQ:(h + 1) * SQ] = results[c]["out"]
    return out


def kernel(Q, K, V, **_unused):
    from concourse.bass_utils import run_bass_kernel_spmd

    nc = _get_nc()
    res = run_bass_kernel_spmd(nc, _shard(Q, K, V), core_ids=list(range(8)))
    return _gather(res.results)


def kernel_traced(Q, K, V, tmpdir=None, **_unused):
    """Like kernel() but with NTFF profiling; returns (out, exec_time_ns)."""
    from concourse.bass_utils import run_bass_kernel_spmd

    nc = _get_nc()
    try:
        res = run_bass_kernel_spmd(
            nc, _shard(Q, K, V), core_ids=list(range(8)), trace=True, tmpdir=tmpdir
        )
    except ModuleNotFoundError:
        res = run_bass_kernel_spmd(nc, _shard(Q, K, V), core_ids=list(range(8)))
    return _gather(res.results), res.exec_time_ns


# revision 15
# speedup vs baseline: 1.1095x; 1.0386x over previous
"""Trainium2 Bass kernel for NeuromodulatedAttention.

Key identity: the reference adds a per-query-row constant (dopamine-serotonin
modulation) to the scores and then standardizes each row ((m - mean)/(std+eps))
before the second softmax.  Adding a row constant shifts the row mean by the
same constant and leaves the row std unchanged, so the modulation cancels
exactly.  Row standardization is also invariant to the 1/sqrt(d) scale except
through the +1e-6 epsilon, which folds into eps' = sqrt(d)*1e-6 applied to the
raw-score std.  The kernel therefore computes:

    S   = Q @ K^T                       (raw scores)
    m   = (S - mean_k(S)) / (std_k(S, ddof=1) + sqrt(d)*1e-6)
    out = softmax_k(m) @ V

Sharding: 8 cores; core c handles batch b=c//2, query rows [h*1024,(h+1)*1024)
with h=c%2.  K/V of the batch are replicated per core; all ops batch-local.

Implementation notes:
 - f32r matmuls (full-rate fp32 path); Q^T/K^T via exact fp32 PE transposes.
 - scores per q-tile live briefly in 4 one-bank PSUM tiles and are evacuated
   to SBUF by ACT copies whose accumulators give the row-sums for free; the
   sumsq comes from one DVE scalar_tensor_tensor per bank.  Banks recycle
   immediately, so the next tiles' QK matmuls overlap the stats chain.
 - rsqrt for the row std: bit-trick seed + 2 Newton steps, fused into
   scalar_tensor_tensor ops; 1/rowsum via fast reciprocal + one Newton step.
 - P = exp(a*S+b) in one ACT pass (per-partition scale/bias, free row-sum);
   P^T via f32r PE transposes in groups of 4 sharing one PSUM bank.
 - Two q-tiles are processed per software-pipeline cycle so their serial
   stats chains interleave on DVE; emission order per cycle is
   [chain+exp(c); QK(c+1); P^T(c); PV(c)] which keeps the PE stream dense.
"""

import numpy as np

B, S, D = 4, 2048, 512
SQ = S // 2            # per-core query rows
P = 128                # partitions
QT_TILES = SQ // P     # 8 q-tiles per core
NPAIR = QT_TILES // 2  # tiles processed in pairs
KC = S // P            # 16 k-chunks
DC = D // P            # 4 d-chunks
NBANK = S // 512       # 4 psum bank-regions for scores

EPS = float(np.float32(np.sqrt(np.float32(512.0)))) * 1e-6  # sqrt(d)*1e-6
C1 = 1.0 / (S - 1)             # 1/2047
C2 = 1.0 / (S * (S - 1.0))    # 1/(2048*2047)
RSQRT_MAGIC = 0x5F3759DF

_CACHE = {}


def _build(use_f32r=True):
    import concourse.bacc as bacc
    import concourse.mybir as mybir
    import concourse.tile as tile
    from concourse.masks import make_identity

    f32 = mybir.dt.float32
    mmdt = mybir.dt.float32r if use_f32r else mybir.dt.float32
    u32 = mybir.dt.uint32
    AF = mybir.ActivationFunctionType
    OP = mybir.AluOpType

    nc = bacc.Bacc(None)
    Qd = nc.dram_tensor("Q", [SQ, D], f32, kind="ExternalInput")
    Kd = nc.dram_tensor("K", [S, D], f32, kind="ExternalInput")
    Vd = nc.dram_tensor("V", [S, D], f32, kind="ExternalInput")
    Od = nc.dram_tensor("out", [SQ, D], f32, kind="ExternalOutput")

    def copy_eng(i):
        return nc.vector if (i % 2 == 0) else nc.scalar

    def do_copy(eng, out_ap, in_ap):
        if eng is nc.vector:
            nc.vector.tensor_copy(out_ap, in_ap)
        else:
            nc.scalar.copy(out_ap, in_ap)

    with tile.TileContext(nc) as tc:
        with (
            tc.tile_pool(name="const", bufs=1) as constp,
            tc.tile_pool(name="ktc", bufs=1) as ktp,
            tc.tile_pool(name="vc", bufs=1) as vp,
            tc.tile_pool(name="stage", bufs=4) as stagep,
            tc.tile_pool(name="qt", bufs=4) as qtp,
            tc.tile_pool(name="pbuf", bufs=3) as pp,
            tc.tile_pool(name="sqp", bufs=1) as sqp,
            tc.tile_pool(name="ptb", bufs=3) as ptp,
            tc.tile_pool(name="osb", bufs=3) as op_,
            tc.tile_pool(name="vec", bufs=6) as vecp,
            tc.tile_pool(name="bank6", bufs=6, space="PSUM") as bk6,
            tc.tile_pool(name="ops", bufs=2, space="PSUM") as ops,
        ):
            ident = constp.tile([P, P], f32)
            make_identity(nc, ident[:])
            ident_r = constp.tile([P, P], mmdt)
            nc.vector.tensor_copy(ident_r[:], ident[:])
            magic = constp.tile([P, 1], u32)
            nc.vector.memset(magic[:], RSQRT_MAGIC)

            # ---- K^T cache, split per 512-wide bank so the first QK matmuls
            # only wait for the first 4 K row-chunks: KTb[b][dlow, dc, k'] ----
            KTb = [
                ktp.tile([P, DC, 512], mmdt, name=f"KT{b}", tag=f"KT{b}")
                for b in range(NBANK)
            ]
            # stage all of K up front: 8 wide DMAs across parallel queues
            Kst = ktp.tile([P, KC, D], f32, name="Kst")
            Kr = Kd.rearrange("(kc kl) d -> kl kc d", kl=P)
            for h in range(8):
                nc.sync.dma_start(
                    Kst[:, 2 * h:2 * h + 2, :], Kr[:, 2 * h:2 * h + 2, :]
                )
            for kc in range(KC):
                tg = bk6.tile([P, DC, P], f32, name="ktg", tag="bk")
                for dc in range(DC):
                    nc.tensor.transpose(
                        tg[:, dc, :], Kst[:, kc, dc * P:(dc + 1) * P], ident[:]
                    )
                ksub = kc % 4
                do_copy(
                    copy_eng(kc),
                    KTb[kc // 4][:, :, ksub * P:(ksub + 1) * P],
                    tg[:],
                )

            # ---- V cache: one SWDGE casting DMA (fp32 -> f32r), no staging ----
            VC = vp.tile([P, KC, D], mmdt)

            def load_v():
                nc.gpsimd.dma_start(
                    VC[:], Vd.rearrange("(kc kl) d -> kl kc d", kl=P)
                )

            ev_t = [None] * QT_TILES     # evacuated scores (SBUF f32)
            s1_t = [None] * QT_TILES     # per-bank row-sum partials [P, NBANK]
            s2_t = [None] * QT_TILES     # per-bank row-sumsq partials
            pt_t = [None] * QT_TILES     # (Pt, rr)
            ptT_t = [None] * QT_TILES    # P^T tiles

            def phase_a1(c):
                """Q^T prep + QK + psum evacuation + sumsq for tiles 2c, 2c+1."""
                qts = []
                for t in (2 * c, 2 * c + 1):
                    qst = stagep.tile([P, D], f32, name="qst", tag="stage")
                    nc.sync.dma_start(qst[:], Qd[t * P:(t + 1) * P, :])
                    QTt = qtp.tile([P, DC, P], mmdt, name="QTt")
                    tg = bk6.tile([P, DC, P], f32, name="qtg", tag="bk")
                    for dc in range(DC):
                        nc.tensor.transpose(
                            tg[:, dc, :], qst[:, dc * P:(dc + 1) * P], ident[:]
                        )
                    do_copy(copy_eng(t), QTt[:], tg[:])
                    qts.append(QTt)

                for ti, t in enumerate((2 * c, 2 * c + 1)):
                    QTt = qts[ti]
                    ev = pp.tile([P, S], f32, name="ev", tag="ev")
                    sqs = sqp.tile([P, S], f32, name="sqscratch", tag="sqscratch")
                    s1p = vecp.tile([P, NBANK], f32, name="s1p")
                    s2p = vecp.tile([P, NBANK], f32, name="s2p")
                    for bk in range(NBANK):
                        bsl = slice(bk * 512, (bk + 1) * 512)
                        sc = bk6.tile([P, 512], f32, name=f"sc{bk}", tag="bk")
                        for dc in range(DC):
                            nc.tensor.matmul(
                                sc[:],
                                QTt[:, dc, :],
                                KTb[bk][:, dc, :],
                                start=(dc == 0),
                                stop=(dc == DC - 1),
                            )
                        nc.scalar.activation(
                            ev[:, bsl], sc[:], AF.Copy,
                            accum_out=s1p[:, bk:bk + 1],
                        )
                        nc.vector.scalar_tensor_tensor(
                            sqs[:, bsl], ev[:, bsl], 1.0, ev[:, bsl],
                            OP.mult, OP.mult, accum_out=s2p[:, bk:bk + 1],
                        )
                    ev_t[t], s1_t[t], s2_t[t] = ev, s1p, s2p

            def phase_a2(c):
                """Interleaved stats chains + exp for tiles 2c, 2c+1."""
                ts = (2 * c, 2 * c + 1)
                v = {}

                def vt(nm, t, dt=f32):
                    tl = vecp.tile([P, 1], dt, name=f"{nm}")
                    v[(nm, t)] = tl
                    return tl[:]

                for t in ts:
                    nc.vector.reduce_sum(
                        vt("s1", t), s1_t[t][:], axis=mybir.AxisListType.X
                    )
                for t in ts:
                    nc.vector.reduce_sum(
                        vt("s2", t), s2_t[t][:], axis=mybir.AxisListType.X
                    )
                for t in ts:  # t1s = (s1*C2)*s1
                    nc.vector.scalar_tensor_tensor(
                        vt("t1s", t), v[("s1", t)][:], C2, v[("s1", t)][:],
                        OP.mult, OP.mult,
                    )
                for t in ts:  # var = s2*C1 - t1s
                    nc.vector.scalar_tensor_tensor(
                        vt("var", t), v[("s2", t)][:], C1, v[("t1s", t)][:],
                        OP.mult, OP.subtract,
                    )
                for t in ts:  # bits = var >> 1
                    nc.vector.tensor_scalar(
                        vt("bits", t, u32), v[("var", t)][:].bitcast(u32), 1, None,
                        OP.logical_shift_right,
                    )
                for t in ts:  # y0 = magic - bits
                    nc.vector.tensor_tensor(
                        vt("y", t).bitcast(u32), magic[:], v[("bits", t)][:],
                        OP.subtract,
                    )
                for it in range(2):  # Newton: y = y*(1.5 - 0.5*var*y*y)
                    for t in ts:
                        if it == 0:
                            vt("t2", t)
                        nc.vector.scalar_tensor_tensor(
                            v[("t2", t)][:], v[("y", t)][:], 1.0, v[("y", t)][:],
                            OP.mult, OP.mult,
                        )
                    for t in ts:
                        nc.vector.scalar_tensor_tensor(
                            v[("t2", t)][:], v[("t2", t)][:], -0.5,
                            v[("var", t)][:], OP.mult, OP.mult,
                        )
                    for t in ts:
                        nc.vector.scalar_tensor_tensor(
                            v[("y", t)][:], v[("t2", t)][:], 1.5, v[("y", t)][:],
                            OP.add, OP.mult,
                        )
                for t in ts:  # t3 = -eps*y^2 ; a = y + t3
                    nc.vector.scalar_tensor_tensor(
                        vt("t3", t), v[("y", t)][:], -EPS, v[("y", t)][:],
                        OP.mult, OP.mult,
                    )
                for t in ts:
                    nc.vector.tensor_tensor(
                        vt("a", t), v[("y", t)][:], v[("t3", t)][:], OP.add
                    )
                for t in ts:  # bb = (s1 * -1/S) * a
                    nc.vector.scalar_tensor_tensor(
                        vt("bb", t), v[("s1", t)][:], -1.0 / S, v[("a", t)][:],
                        OP.mult, OP.mult,
                    )

                for t in ts:
                    Pt = pp.tile([P, S], mmdt, name="Pt", tag="Pt")
                    rs = vt("rs", t)
                    nc.scalar.activation(
                        Pt[:], ev_t[t][:], AF.Exp,
                        bias=v[("bb", t)][:], scale=v[("a", t)][:],
                        accum_out=rs,
                    )
                    rr = vt("rr", t)
                    scr = vt("scr", t)
                    nc.vector.reciprocal_approx_fast(out=scr, in_=rs)
                    # one Newton step: rr = (2 - rs*scr) * scr
                    rn = vt("rn", t)
                    nc.vector.scalar_tensor_tensor(
                        rn, rs, 1.0, scr, OP.mult, OP.mult
                    )
                    nc.vector.tensor_scalar(rn, rn, -1.0, 2.0, OP.mult, OP.add)
                    nc.vector.tensor_tensor(rr, rn, scr, OP.mult)
                    pt_t[t] = (Pt, rr)

            def phase_b1(c):
                for t in (2 * c, 2 * c + 1):
                    Pt, _ = pt_t[t]
                    PT = ptp.tile([P, KC, P], mmdt, name="PT")
                    for g in range(KC // DC):
                        tg2 = bk6.tile([P, DC, P], mmdt, name="ptg", tag="bk")
                        for j in range(DC):
                            kc = g * DC + j
                            nc.tensor.transpose(
                                tg2[:, j, :], Pt[:, kc * P:(kc + 1) * P],
                                ident_r[:],
                            )
                        do_copy(
                            copy_eng(g + t), PT[:, g * DC:(g + 1) * DC, :], tg2[:]
                        )
                    ptT_t[t] = PT

            def phase_b2(c):
                for t in (2 * c, 2 * c + 1):
                    _, rr = pt_t[t]
                    PT = ptT_t[t]
                    ot = ops.tile([P, D], f32, name="ot")
                    for kc in range(KC):
                        nc.tensor.matmul(
                            ot[:],
                            PT[:, kc, :],
                            VC[:, kc, :],
                            start=(kc == 0),
                            stop=(kc == KC - 1),
                        )
                    osb = op_.tile([P, D], f32, name="osb")
                    nc.scalar.mul(osb[:], ot[:], rr[:])
                    nc.sync.dma_start(Od[t * P:(t + 1) * P, :], osb[:])

            phase_a1(0)
            load_v()
            for c in range(NPAIR):
                phase_a2(c)
                if c + 1 < NPAIR:
                    phase_a1(c + 1)
                phase_b1(c)
                phase_b2(c)

    nc.compile()
    return nc


def _get_nc():
    if "nc" not in _CACHE:
        _CACHE["nc"] = _build(use_f32r=True)
    return _CACHE["nc"]


def _shard(Q, K, V):
    Q = np.ascontiguousarray(np.asarray(Q, dtype=np.float32))
    K = np.ascontiguousarray(np.asarray(K, dtype=np.float32))
    V = np.ascontiguousarray(np.asarray(V, dtype=np.float32))
    in_maps = []
    for c in range(8):
        b, h = divmod(c, 2)
        in_maps.append(
            {
                "Q": np.ascontiguousarray(Q[b, h * SQ:(h + 1) * SQ]),
                "K": np.ascontiguousarray(K[b]),
                "V": np.ascontiguousarray(V[b]),
            }
        )
    return in_maps


def _gather(results):
    out = np.empty((B, S, D), np.float32)
    for c in range(8):
        b, h = divmod(c, 2)
        out[b, h * SQ:(h + 1) * SQ] = results[c]["out"]
    return out


def kernel(Q, K, V, **_unused):
    from concourse.bass_utils import run_bass_kernel_spmd

    nc = _get_nc()
    res = run_bass_kernel_spmd(nc, _shard(Q, K, V), core_ids=list(range(8)))
    return _gather(res.results)


def kernel_traced(Q, K, V, tmpdir=None, **_unused):
    """Like kernel() but with NTFF profiling; returns (out, exec_time_ns)."""
    from concourse.bass_utils import run_bass_kernel_spmd

    nc = _get_nc()
    try:
        res = run_bass_kernel_spmd(
            nc, _shard(Q, K, V), core_ids=list(range(8)), trace=True, tmpdir=tmpdir
        )
    except ModuleNotFoundError:
        res = run_bass_kernel_spmd(nc, _shard(Q, K, V), core_ids=list(range(8)))
    return _gather(res.results), res.exec_time_ns
